# revision 16
# baseline (speedup 1.0000x reference)
"""Trainium2 Bass kernel for CombinedAdvancedLoss (focal + contrastive +
circularity + consensus), data-parallel over 8 NeuronCores.

Sharding: batch dim B=32 -> 4 items per core for logits/target/masks/
method_preds. features (1024x512) are passed to each core TRANSPOSED and
ROLLED by -core*128 rows, so every core computes the same SPMD program on
"its" 128 rows of the 1024x1024 similarity matrix (the diagonal lands in
local column block 0 and the positive pair in block 4).

Engine split per core (all inputs pre-cast to bf16 on host):
  ScalarE: exp(logits), ln(S), p=exp(-ce), |row-diff| accum, norm chain
  VectorE: one-hot masks + masked-select chain, pair products, col-diffs
  TensorE: S=sum_c exp via identity-matmul PSUM accumulation; large
           free-axis sums via ones-matmuls routed into a PSUM accumulator;
           feature Gram matrix; row-diff stencil matmuls
Each core emits two small partial vectors; the host combines them (the only
nonlinear cross-core math - IoU ratios, circularity - acts on a few scalars).
"""

import sys

for _p in ("/opt/trn_rl_repo",):
    if _p not in sys.path:
        sys.path.insert(0, _p)

import numpy as np
import ml_dtypes

import concourse.bass as bass
import concourse.tile as tile
from concourse import mybir
from concourse.bass_utils import run_bass_kernel_spmd

import bass_rust as _bass_rust

# ---------------------------------------------------------------------------
# The walrus build in this container rejects >2 sync waits per instruction.
# Post-pass: hoist excess waits onto inserted same-engine NoOps.
_WAIT_CAP = 1


def _split_sync_waits(nc):
    n = 0
    for fn in nc.m.functions:
        for blk in fn.blocks:
            insts = blk.instructions
            i = 0
            while i < len(insts):
                inst = insts[i]
                si = inst.sync_info
                if si is not None and len(si.on_wait) > _WAIT_CAP:
                    waits = list(si.on_wait)
                    keep = waits[-_WAIT_CAP:]
                    extra = waits[:-_WAIT_CAP]
                    nops = []
                    for j in range(0, len(extra), _WAIT_CAP):
                        nop = mybir.InstNoOp(
                            name=f"I-wsplit-{n}", engine=inst.engine)
                        n += 1
                        nop.sync_info = _bass_rust.SyncInfo(
                            on_wait=extra[j:j + _WAIT_CAP], on_update=[])
                        nops.append(nop)
                    inst.sync_info = _bass_rust.SyncInfo(
                        on_wait=keep, on_update=list(si.on_update))
                    for k, nop in enumerate(nops):
                        insts.insert(i + k, nop)
                    i += len(nops)
                i += 1
# ---------------------------------------------------------------------------

F32 = mybir.dt.float32
BF16 = mybir.dt.bfloat16
FP8 = mybir.dt.float8e4
U8 = mybir.dt.uint8
AF = mybir.ActivationFunctionType
OP = mybir.AluOpType
AX = mybir.AxisListType

NCORES = 8
B, C, H, W = 32, 8, 256, 256
BP = B // NCORES          # batch items per core (4)
HW = H * W                # 65536
FD = BP * HW // 128       # free dim of a full-core tile (2048)
BF, DF = 1024, 512        # features shape
TEMP = 0.07
GAMMA_SCALE = 0.25        # ALPHA (0.25 for every class) * W_FOCAL

# SBUF acc [128, NACC] columns (partition-wise partials; summed via ones-MM)
NACC = 32
K_FOCAL = 0               # TTR: sum 0.25*(1-p)^2 * ce
K_CONTRAST = 1            # per-row lse - pos
K_EX = 2                  # 4 cols: per-b |row-diff| (within chunks)
K_EY = 6                  # 8 cols: per-(b,chunk) |col-diff|
K_BND = 14                # 4 cols: per-b boundary |m[128]-m[127]| (part 0)

# PSUM accP [32, 1024] rows (free-axis sums via routed ones-MMs)
R_AREA = 0                # 4 rows: per-b mask area
R_S = 4                   # 3 rows: per-method sum of preds
R_I = 7                   # 3 rows: per-pair sum pi*pj (01, 02, 12)
R_W = 10                  # 1 row: sum (1-p)^2 * ce (x0.25 on host)
NROW = 32


def _build_nc():
    nc = bass.Bass()

    lg = nc.declare_dram_parameter("lg", [C, 128, FD], FP8, isOutput=False)
    tg = nc.declare_dram_parameter("tg", [128, FD], BF16, isOutput=False)
    mk = nc.declare_dram_parameter("mk", [128, BP, 2, 256], BF16, isOutput=False)
    mb = nc.declare_dram_parameter("mb", [1, BP, 2, 256], BF16, isOutput=False)
    mp = nc.declare_dram_parameter("mp", [3, 128, FD], BF16, isOutput=False)
    ft = nc.declare_dram_parameter("ft", [128, 4, BF], BF16, isOutput=False)
    idb = nc.declare_dram_parameter("idb", [128, 128], BF16, isOutput=False)
    zmb = nc.declare_dram_parameter("zmb", [128, 128], BF16, isOutput=False)
    ohb = nc.declare_dram_parameter("ohb", [128, 63], BF16, isOutput=False)
    onesr = nc.declare_dram_parameter("onesr", [1, 128], BF16, isOutput=False)
    pa = nc.declare_dram_parameter("pa", [1, NACC], F32, isOutput=True)
    pb = nc.declare_dram_parameter("pb", [NROW, 1], F32, isOutput=True)

    with tile.TileContext(nc) as tc:
        _emit(nc, tc, lg, tg, mk, mb, mp, ft, idb, zmb, ohb, onesr, pa, pb)
    _split_sync_waits(nc)
    return nc


def _emit(nc, tc, lg, tg, mk, mb, mp, ft, idb, zmb, ohb, onesr, pa, pb):
    from contextlib import ExitStack

    ctx = ExitStack()
    with ctx:
        singles = ctx.enter_context(tc.tile_pool(name="singles", bufs=1))
        lpool = ctx.enter_context(tc.tile_pool(name="lpool", bufs=8))
        qpool = ctx.enter_context(tc.tile_pool(name="qpool", bufs=3))
        mpool = ctx.enter_context(tc.tile_pool(name="mpool", bufs=2))
        selpool = ctx.enter_context(tc.tile_pool(name="selpool", bufs=2))
        tpool = ctx.enter_context(tc.tile_pool(name="tpool", bufs=2))
        sqpool = ctx.enter_context(tc.tile_pool(name="sqpool", bufs=2))
        prodpool = ctx.enter_context(tc.tile_pool(name="prodpool", bufs=2))
        scratch = ctx.enter_context(tc.tile_pool(name="scratch", bufs=1))
        tiny = ctx.enter_context(tc.tile_pool(name="tiny", bufs=1))
        pAcc = ctx.enter_context(
            tc.tile_pool(name="pAcc", bufs=1, space="PSUM"))

        # ---------- DMA order: logits first (they pace the exp chain) ----
        id_t = singles.tile([128, 128], BF16)
        nc.sync.dma_start(out=id_t, in_=idb[:, :])
        l_tiles = []
        for c in range(2):
            l_c = lpool.tile([128, FD], FP8, tag="l", name=f"l{c}")
            nc.sync.dma_start(out=l_c, in_=lg[c])
            l_tiles.append(l_c)
        tg_t = singles.tile([128, FD], BF16)
        nc.sync.dma_start(out=tg_t, in_=tg[:, :])
        oh_t = singles.tile([128, 63], BF16)
        nc.sync.dma_start(out=oh_t, in_=ohb[:, :])
        zm_t = singles.tile([128, 128], BF16)
        nc.sync.dma_start(out=zm_t, in_=zmb[:, :])
        ones_r = singles.tile([1, 128], BF16)
        nc.sync.dma_start(out=ones_r, in_=onesr[:, :])
        for c in range(2, 4):
            l_c = lpool.tile([128, FD], FP8, tag="l", name=f"l{c}")
            nc.sync.dma_start(out=l_c, in_=lg[c])
            l_tiles.append(l_c)
        ft_t = singles.tile([128, 4, BF], BF16)
        nc.sync.dma_start(out=ft_t, in_=ft[:, :, :])
        l_c = lpool.tile([128, FD], FP8, tag="l", name="l4")
        nc.sync.dma_start(out=l_c, in_=lg[4])
        l_tiles.append(l_c)
        mk_t = singles.tile([128, BP, 2, 256], BF16)
        nc.sync.dma_start(out=mk_t, in_=mk[:, :, :, :])
        l_c = lpool.tile([128, FD], FP8, tag="l", name="l5")
        nc.sync.dma_start(out=l_c, in_=lg[5])
        l_tiles.append(l_c)
        mp_t = [
            singles.tile([128, FD], BF16, name=f"mp{i}") for i in range(3)
        ]
        nc.sync.dma_start(out=mp_t[0], in_=mp[0])
        nc.sync.dma_start(out=mp_t[1], in_=mp[1])
        for c in range(6, 8):
            l_c = lpool.tile([128, FD], FP8, tag="l", name=f"l{c}")
            nc.sync.dma_start(out=l_c, in_=lg[c])
            l_tiles.append(l_c)
        nc.sync.dma_start(out=mp_t[2], in_=mp[2])
        mb_t = singles.tile([1, BP, 2, 256], BF16)
        nc.sync.dma_start(out=mb_t, in_=mb[:, :, :, :])

        acc = singles.tile([128, NACC], F32)
        nc.gpsimd.memset(acc, 0.0)
        onesf = singles.tile([128, 1], F32)
        nc.gpsimd.memset(onesf, 1.0)

        accP = pAcc.tile([NROW, 512], F32)

        gs = scratch.tile([128, BF], BF16, tag="gs")
        rnb_sb = scratch.tile([128, BF], BF16, tag="rnb")

        # ---------- phase A: focal loop + bank-slot side work ----------
        t_acc = None
        with tc.tile_pool(name="pS", bufs=1, space="PSUM") as pS, \
             tc.tile_pool(name="pSS", bufs=1, space="PSUM") as pSS, \
             tc.tile_pool(name="pMix", bufs=1, space="PSUM") as pMix:
            sP = [pS.tile([128, 512], F32, tag=f"s{h}", name=f"s{h}")
                  for h in range(4)]
            ssP = pSS.tile([1, BF], F32)

            # HAM warm-up: dense dummy matmuls so the PE clock-gate opens
            # before the real S accumulation begins.
            for wu in range(36):
                nc.tensor.matmul(
                    out=sP[wu % 4][:, 0:128], lhsT=id_t, rhs=id_t, start=True,
                    stop=True, skip_group_check=True,
                )

            for c in range(C):
                l_c = l_tiles[c]
                q_c = qpool.tile([128, FD], BF16, tag="q")
                nc.scalar.activation(out=q_c, in_=l_c, func=AF.Exp)
                for h in range(4):
                    nc.tensor.matmul(
                        out=sP[h], lhsT=id_t,
                        rhs=q_c[:, h * 512:(h + 1) * 512],
                        start=(c == 0), stop=(c == C - 1),
                    )
                if c == 0:
                    t_acc = scratch.tile([128, FD], BF16, tag="t")
                    nc.vector.tensor_copy(out=t_acc, in_=q_c)
                else:
                    m_c = mpool.tile([128, FD], U8, tag="m")
                    nc.vector.tensor_scalar(
                        out=m_c, in0=tg_t, scalar1=float(c), scalar2=None,
                        op0=OP.is_equal,
                    )
                    nc.vector.copy_predicated(
                        out=t_acc, mask=m_c, data=q_c)

            # -- side work that shares the one spare PSUM bank (pMix) --
            # norms: ss = sum_d ft^2 (squares on GPSIMD, sums on PE)
            for dc in range(4):
                sq = sqpool.tile([128, BF], BF16, tag="sq")
                nc.vector.tensor_tensor(
                    out=sq, in0=ft_t[:, dc], in1=ft_t[:, dc], op=OP.mult)
                for h in range(2):
                    nc.tensor.matmul(
                        out=ssP[:, h * 512:(h + 1) * 512],
                        lhsT=oh_t[:, 31:32],
                        rhs=sq[:, h * 512:(h + 1) * 512],
                        start=(dc == 0), stop=(dc == 3),
                        skip_group_check=True,
                    )
            lnss = tiny.tile([1, BF], F32, tag="lnss")
            nc.scalar.activation(out=lnss, in_=ssP, func=AF.Ln)
            rnrow = tiny.tile([1, BF], BF16, tag="rnrow")
            nc.scalar.activation(out=rnrow, in_=lnss, func=AF.Exp, scale=-0.5)

            # rnb = broadcast of rnrow to 128 partitions (rank-1 matmul)
            for h in range(2):
                rP = pMix.tile([128, 512], F32, tag="mix")
                nc.tensor.matmul(
                    out=rP, lhsT=ones_r, rhs=rnrow[:, h * 512:(h + 1) * 512],
                    start=True, stop=True)
                nc.scalar.activation(
                    out=rnb_sb[:, h * 512:(h + 1) * 512], in_=rP,
                    func=AF.Copy)
            # Gram: G = ftT.T @ ftT accumulated over 4 d-chunks, per half
            for h in range(2):
                gP = pMix.tile([128, 512], F32, tag="mix")
                for dc in range(4):
                    nc.tensor.matmul(
                        out=gP, lhsT=ft_t[:, dc, 0:128],
                        rhs=ft_t[:, dc, h * 512:(h + 1) * 512],
                        start=(dc == 0), stop=(dc == 3),
                    )
                nc.vector.tensor_tensor(
                    out=gs[:, h * 512:(h + 1) * 512], in0=gP,
                    in1=rnb_sb[:, h * 512:(h + 1) * 512], op=OP.mult)
            # circ row-diffs: stencil matmul + |.| accumulate on ScalarE
            junkC = scratch.tile([128, 512], BF16, tag="junkC")
            for b in range(BP):
                zP = pMix.tile([128, 512], F32, tag="mix")
                nc.tensor.matmul(
                    out=zP, lhsT=zm_t,
                    rhs=mk_t[:, b].rearrange("p c w -> p (c w)"),
                    start=True, stop=True,
                )
                nc.scalar.activation(
                    out=junkC, in_=zP, func=AF.Abs,
                    accum_out=acc[:, K_EX + b:K_EX + b + 1],
                )

            # lnS from the S PSUM banks (last: frees pS on scope exit)
            lns = scratch.tile([128, FD], BF16, tag="lns")
            for h in range(4):
                nc.scalar.activation(
                    out=lns[:, h * 512:(h + 1) * 512], in_=sP[h], func=AF.Ln)

        # ---------- contrastive tail (SBUF only) ----------
        rdj = scratch.tile([128, 128], BF16, tag="rdj")
        nc.vector.tensor_tensor(
            out=rdj, in0=rnb_sb[:, 0:128], in1=id_t, op=OP.mult)
        rn_r = tiny.tile([128, 1], F32, tag="rnr")
        nc.vector.tensor_reduce(
            out=rn_r, in_=rdj, axis=AX.X, op=OP.add)
        gq = scratch.tile([128, BF], BF16, tag="gq")
        nc.vector.tensor_scalar(
            out=gq, in0=gs, scalar1=rn_r, scalar2=None, op0=OP.mult)
        pj = scratch.tile([128, 128], BF16, tag="pj")
        nc.vector.tensor_tensor(
            out=pj, in0=gq[:, 512:640], in1=id_t, op=OP.mult)
        posu = tiny.tile([128, 1], F32, tag="posu")
        nc.vector.tensor_reduce(out=posu, in_=pj, axis=AX.X, op=OP.add)
        nc.vector.scalar_tensor_tensor(
            out=gq[:, 0:128], in0=id_t, scalar=-1e4, in1=gq[:, 0:128],
            op0=OP.mult, op1=OP.add,
        )
        esum = tiny.tile([128, 1], F32, tag="esum")
        junkB = scratch.tile([128, BF], BF16, tag="junkB")
        nc.scalar.activation(
            out=junkB, in_=gq, func=AF.Exp, scale=1.0 / TEMP, accum_out=esum)
        lse = tiny.tile([128, 1], F32, tag="lse")
        nc.scalar.activation(out=lse, in_=esum, func=AF.Ln)
        nc.vector.scalar_tensor_tensor(
            out=acc[:, K_CONTRAST:K_CONTRAST + 1], in0=posu,
            scalar=-1.0 / TEMP, in1=lse, op0=OP.mult, op1=OP.add,
        )

        # ---------- circularity col-diffs + boundary (GPSIMD) ----------
        d_y = scratch.tile([128, BP, 2, 255], BF16, tag="dy")
        nc.vector.tensor_tensor(
            out=d_y, in0=mk_t[:, :, :, 1:256], in1=mk_t[:, :, :, 0:255],
            op=OP.subtract,
        )
        junkE = scratch.tile([128, 512], BF16, tag="junkE")
        for b in range(BP):
            nc.scalar.activation(
                out=junkE[:, 0:510].rearrange("p (c w) -> p c w", c=2),
                in_=d_y[:, b], func=AF.Abs,
                accum_out=acc[:, K_EY + b:K_EY + b + 1],
            )
        d_b = tiny.tile([1, BP, 256], BF16, tag="db")
        nc.vector.tensor_tensor(
            out=d_b, in0=mb_t[:, :, 1], in1=mb_t[:, :, 0], op=OP.subtract)
        nc.vector.tensor_reduce(
            out=acc[0:1, K_BND:K_BND + BP], in_=d_b,
            axis=AX.X, op=OP.add, apply_absolute_value=True,
        )

        # ---------- focal tail ----------
        lnt = scratch.tile([128, FD], BF16, tag="lnt")
        nc.scalar.activation(out=lnt, in_=t_acc, func=AF.Ln)
        ce = scratch.tile([128, FD], BF16, tag="ce")
        nc.vector.tensor_tensor(out=ce, in0=lns, in1=lnt, op=OP.subtract)
        p_t = scratch.tile([128, FD], BF16, tag="p")
        nc.scalar.activation(out=p_t, in_=ce, func=AF.Exp, scale=-1.0)
        v_t = scratch.tile([128, FD], BF16, tag="v")
        nc.scalar.activation(
            out=v_t, in_=p_t, func=AF.Square, scale=-1.0, bias=1.0)
        w_t = scratch.tile([128, FD], BF16, tag="w")
        nc.vector.tensor_tensor(out=w_t, in0=v_t, in1=ce, op=OP.mult)

        # ---------- consensus + w sums (PE ones-MMs into accP rows) -----
        first = [True]

        def accmm(q, rhs, stop=False):
            n = rhs.free_size()
            chunks = [(h, min(512, n - h)) for h in range(0, n, 512)]
            for ci, (h, w) in enumerate(chunks):
                nc.tensor.matmul(
                    out=accP[:, 0:w],
                    lhsT=oh_t[:, 31 - q:63 - q],
                    rhs=rhs[:, h:h + w],
                    start=first[0],
                    stop=stop and ci == len(chunks) - 1,
                    skip_group_check=True,
                )
                first[0] = False

        # first accP matmul is full-width so every accumulator element
        # gets its has_written bit set before narrower adds.
        for i in range(3):
            accmm(R_S + i, mp_t[i])
        for b in range(BP):
            accmm(R_AREA + b, mk_t[:, b].rearrange("p c w -> p (c w)"))
        for k, (i, j) in enumerate(((0, 1), (0, 2), (1, 2))):
            pr = prodpool.tile([128, FD], BF16, tag="prod")
            nc.vector.tensor_tensor(
                out=pr, in0=mp_t[i], in1=mp_t[j], op=OP.mult)
            accmm(R_I + k, pr)
        accmm(R_W, w_t, stop=True)

        # ---------- finalize ----------
        with tc.tile_pool(name="pF", bufs=1, space="PSUM") as pF:
            junkD = scratch.tile([NROW, 512], BF16, tag="junkD")
            pb_sb = tiny.tile([NROW, 1], F32, tag="pbs")
            nc.scalar.activation(
                out=junkD, in_=accP, func=AF.Copy, accum_out=pb_sb)
            nc.sync.dma_start(out=pb[:, :], in_=pb_sb)

            pfin = pF.tile([1, NACC], F32)
            nc.tensor.matmul(
                out=pfin, lhsT=onesf, rhs=acc, start=True, stop=True)
            pa_sb = tiny.tile([1, NACC], F32, tag="pas")
            nc.vector.tensor_copy(out=pa_sb, in_=pfin)
            nc.sync.dma_start(out=pa[:, :], in_=pa_sb)


def _zmat():
    ident = np.eye(128, dtype=np.float32)
    z = np.roll(ident, -1, axis=0) - ident
    z[:, 127] = 0.0
    return np.ascontiguousarray(z)


def _host_inputs(logits, target, features, masks, method_preds):
    """Slice/reshape/cast full inputs into per-core input maps (bf16)."""
    bf = ml_dtypes.bfloat16
    ident = np.eye(128, dtype=np.float32)
    ohb = np.zeros((128, 63), dtype=np.float32)
    ohb[:, 31] = 1.0
    consts = {
        "idb": ident.astype(bf),
        "zmb": _zmat().astype(bf),
        "ohb": ohb.astype(bf),
        "onesr": np.ones((1, 128), dtype=np.float32).astype(bf),
    }
    in_maps = []
    for c in range(NCORES):
        b0 = c * BP
        lgc = (logits[b0:b0 + BP].reshape(BP, C, 128, 512)
               .transpose(1, 2, 0, 3).reshape(C, 128, FD))
        tgc = (target[b0:b0 + BP].reshape(BP, 128, 512)
               .transpose(1, 0, 2).reshape(128, FD))
        mkc = (masks[b0:b0 + BP, 0].reshape(BP, 2, 128, 256)
               .transpose(2, 0, 1, 3))
        mbc = masks[b0:b0 + BP, 0, 127:129, :].reshape(1, BP, 2, 256)
        mpc = (method_preds[:, b0:b0 + BP].reshape(3, BP, 128, 512)
               .transpose(0, 2, 1, 3).reshape(3, 128, FD))
        ftc = (np.roll(features, -c * 128, axis=0).T
               .reshape(4, 128, BF).transpose(1, 0, 2))
        in_maps.append({
            "lg": np.ascontiguousarray(lgc).astype(ml_dtypes.float8_e4m3fn),
            "tg": np.ascontiguousarray(tgc.astype(np.float32)).astype(bf),
            "mk": np.ascontiguousarray(mkc).astype(bf),
            "mb": np.ascontiguousarray(mbc).astype(bf),
            "mp": np.ascontiguousarray(mpc).astype(bf),
            "ft": np.ascontiguousarray(ftc).astype(bf),
            **consts,
        })
    return in_maps


def _combine(pas, pbs):
    """Host-side combination of the per-core partial vectors."""
    PA = np.stack([np.asarray(p).reshape(-1).astype(np.float64)
                   for p in pas])  # [8, NACC]
    PB = np.stack([np.asarray(p).reshape(-1).astype(np.float64)
                   for p in pbs])  # [8, NROW]

    focal = GAMMA_SCALE * PB[:, R_W].sum() / (B * HW)
    contrast = 0.5 * PA[:, K_CONTRAST].sum() / BF

    circ_total = 0.0
    for c in range(NCORES):
        for b in range(BP):
            area = PB[c, R_AREA + b]
            ex = PA[c, K_EX + b] + PA[c, K_BND + b]
            ey = PA[c, K_EY + b]
            per = ex + ey
            if area > 0 and per > 0:
                circv = 4.0 * np.pi * area / max(per, 1e-12) ** 2
                circ_total += (circv - 1.0) ** 2
    circ = 0.1 * circ_total / B

    S = PB[:, R_S:R_S + 3].sum(axis=0)
    I = PB[:, R_I:R_I + 3].sum(axis=0)
    cons_total = 0.0
    for k, (i, j) in enumerate(((0, 1), (0, 2), (1, 2))):
        union = S[i] + S[j] - I[k]
        iou = I[k] / (union + 1e-6)
        cons_total += max(0.6 - iou, 0.0)
    consensus = 0.3 * cons_total / 3.0

    return np.float32(focal + contrast + circ + consensus)


_CACHED_NC = None


def _get_nc():
    global _CACHED_NC
    if _CACHED_NC is None:
        _CACHED_NC = _build_nc()
    return _CACHED_NC


def kernel(logits, target, features, masks, method_preds):
    logits = np.asarray(logits, dtype=np.float32)
    target = np.asarray(target, dtype=np.int32)
    features = np.asarray(features, dtype=np.float32)
    masks = np.asarray(masks, dtype=np.float32)
    method_preds = np.asarray(method_preds, dtype=np.float32)

    in_maps = _host_inputs(logits, target, features, masks, method_preds)
    res = run_bass_kernel_spmd(_get_nc(), in_maps, list(range(NCORES)))
    pas = [res.results[c]["pa"] for c in range(NCORES)]
    pbs = [res.results[c]["pb"] for c in range(NCORES)]
    return _combine(pas, pbs)


# revision 20
# speedup vs baseline: 1.1024x; 1.1024x over previous
"""Trainium2 Bass kernel for CombinedAdvancedLoss (focal + contrastive +
circularity + consensus), data-parallel over 8 NeuronCores.

Sharding: batch dim B=32 -> 4 items per core for logits/target/masks/
method_preds. features (1024x512) are passed to each core TRANSPOSED and
ROLLED by -core*128 rows, so every core computes the same SPMD program on
"its" 128 rows of the 1024x1024 similarity matrix (the diagonal lands in
local column block 0 and the positive pair in block 4).

Engine split per core (all inputs pre-cast to bf16 on host):
  ScalarE: exp(logits), ln(S), p=exp(-ce), |row-diff| accum, norm chain
  VectorE: one-hot masks + masked-select chain, pair products, col-diffs
  TensorE: S=sum_c exp via identity-matmul PSUM accumulation; large
           free-axis sums via ones-matmuls routed into a PSUM accumulator;
           feature Gram matrix; row-diff stencil matmuls
Each core emits two small partial vectors; the host combines them (the only
nonlinear cross-core math - IoU ratios, circularity - acts on a few scalars).
"""

import sys

for _p in ("/opt/trn_rl_repo",):
    if _p not in sys.path:
        sys.path.insert(0, _p)

import numpy as np
import ml_dtypes

import concourse.bass as bass
import concourse.tile as tile
from concourse import mybir
from concourse.bass_utils import run_bass_kernel_spmd

import bass_rust as _bass_rust

# ---------------------------------------------------------------------------
# The walrus build in this container rejects >2 sync waits per instruction.
# Post-pass: hoist excess waits onto inserted same-engine NoOps.
_WAIT_CAP = 1


def _split_sync_waits(nc):
    n = 0
    for fn in nc.m.functions:
        for blk in fn.blocks:
            insts = blk.instructions
            i = 0
            while i < len(insts):
                inst = insts[i]
                si = inst.sync_info
                if si is not None and len(si.on_wait) > _WAIT_CAP:
                    waits = list(si.on_wait)
                    keep = waits[-_WAIT_CAP:]
                    extra = waits[:-_WAIT_CAP]
                    nops = []
                    for j in range(0, len(extra), _WAIT_CAP):
                        nop = mybir.InstNoOp(
                            name=f"I-wsplit-{n}", engine=inst.engine)
                        n += 1
                        nop.sync_info = _bass_rust.SyncInfo(
                            on_wait=extra[j:j + _WAIT_CAP], on_update=[])
                        nops.append(nop)
                    inst.sync_info = _bass_rust.SyncInfo(
                        on_wait=keep, on_update=list(si.on_update))
                    for k, nop in enumerate(nops):
                        insts.insert(i + k, nop)
                    i += len(nops)
                i += 1
# ---------------------------------------------------------------------------

F32 = mybir.dt.float32
BF16 = mybir.dt.bfloat16
FP8 = mybir.dt.float8e4
U8 = mybir.dt.uint8
AF = mybir.ActivationFunctionType
OP = mybir.AluOpType
AX = mybir.AxisListType

NCORES = 8
B, C, H, W = 32, 8, 256, 256
BP = B // NCORES          # batch items per core (4)
HW = H * W                # 65536
FD = BP * HW // 128       # free dim of a full-core tile (2048)
BF, DF = 1024, 512        # features shape
TEMP = 0.07
GAMMA_SCALE = 0.25        # ALPHA (0.25 for every class) * W_FOCAL

# SBUF acc [128, NACC] columns (partition-wise partials; summed via ones-MM)
NACC = 32
K_FOCAL = 0               # TTR: sum 0.25*(1-p)^2 * ce
K_CONTRAST = 1            # per-row lse - pos
K_EX = 2                  # 4 cols: per-b |row-diff| (within chunks)
K_EY = 6                  # 8 cols: per-(b,chunk) |col-diff|
K_BND = 14                # 4 cols: per-b boundary |m[128]-m[127]| (part 0)

# PSUM accP [32, 1024] rows (free-axis sums via routed ones-MMs)
R_AREA = 0                # 4 rows: per-b mask area
R_S = 4                   # 3 rows: per-method sum of preds
R_I = 7                   # 3 rows: per-pair sum pi*pj (01, 02, 12)
R_W = 10                  # 1 row: sum (1-p)^2 * ce (x0.25 on host)
NROW = 32


def _build_nc():
    nc = bass.Bass()

    lg = nc.declare_dram_parameter("lg", [C, 128, FD], FP8, isOutput=False)
    tg = nc.declare_dram_parameter("tg", [128, FD], BF16, isOutput=False)
    mk = nc.declare_dram_parameter("mk", [128, BP, 2, 256], BF16, isOutput=False)
    mb = nc.declare_dram_parameter("mb", [1, BP, 2, 256], BF16, isOutput=False)
    mp = nc.declare_dram_parameter("mp", [3, 128, FD], BF16, isOutput=False)
    ft = nc.declare_dram_parameter("ft", [128, 4, BF], BF16, isOutput=False)
    idb = nc.declare_dram_parameter("idb", [128, 128], BF16, isOutput=False)
    zmb = nc.declare_dram_parameter("zmb", [128, 128], BF16, isOutput=False)
    ohb = nc.declare_dram_parameter("ohb", [128, 63], BF16, isOutput=False)
    onesr = nc.declare_dram_parameter("onesr", [1, 128], BF16, isOutput=False)
    pa = nc.declare_dram_parameter("pa", [1, NACC], F32, isOutput=True)
    pb = nc.declare_dram_parameter("pb", [NROW, 1], F32, isOutput=True)

    with tile.TileContext(nc) as tc:
        _emit(nc, tc, lg, tg, mk, mb, mp, ft, idb, zmb, ohb, onesr, pa, pb)
    _split_sync_waits(nc)
    return nc


def _emit(nc, tc, lg, tg, mk, mb, mp, ft, idb, zmb, ohb, onesr, pa, pb):
    from contextlib import ExitStack

    ctx = ExitStack()
    with ctx:
        singles = ctx.enter_context(tc.tile_pool(name="singles", bufs=1))
        lpool = ctx.enter_context(tc.tile_pool(name="lpool", bufs=8))
        qpool = ctx.enter_context(tc.tile_pool(name="qpool", bufs=3))
        mpool = ctx.enter_context(tc.tile_pool(name="mpool", bufs=8))
        selpool = ctx.enter_context(tc.tile_pool(name="selpool", bufs=2))
        tpool = ctx.enter_context(tc.tile_pool(name="tpool", bufs=2))
        sqpool = ctx.enter_context(tc.tile_pool(name="sqpool", bufs=2))
        prodpool = ctx.enter_context(tc.tile_pool(name="prodpool", bufs=2))
        scratch = ctx.enter_context(tc.tile_pool(name="scratch", bufs=1))
        tiny = ctx.enter_context(tc.tile_pool(name="tiny", bufs=1))
        pAcc = ctx.enter_context(
            tc.tile_pool(name="pAcc", bufs=1, space="PSUM"))

        # ---------- DMA order: logits first (they pace the exp chain) ----
        l_tiles = []

        def lgdma(c):
            t = lpool.tile([128, FD], FP8, tag="l", name=f"l{c}")
            nc.sync.dma_start(out=t, in_=lg[c])
            l_tiles.append(t)

        lgdma(0)
        id_t = singles.tile([128, 128], BF16)
        nc.sync.dma_start(out=id_t, in_=idb[:, :])
        lgdma(1)
        tg_t = singles.tile([128, FD], BF16)
        nc.sync.dma_start(out=tg_t, in_=tg[:, :])
        mk_t = singles.tile([128, BP, 2, 256], BF16)
        nc.sync.dma_start(out=mk_t, in_=mk[:, :, :, :])
        zm_t = singles.tile([128, 128], BF16)
        nc.sync.dma_start(out=zm_t, in_=zmb[:, :])
        lgdma(2)
        ft_t = singles.tile([128, 4, BF], BF16)
        nc.sync.dma_start(out=ft_t, in_=ft[:, :, :])
        oh_t = singles.tile([128, 63], BF16)
        nc.sync.dma_start(out=oh_t, in_=ohb[:, :])
        lgdma(3)
        ones_r = singles.tile([1, 128], BF16)
        nc.sync.dma_start(out=ones_r, in_=onesr[:, :])
        for c in range(4, C):
            lgdma(c)
        mp_t = [
            singles.tile([128, FD], BF16, name=f"mp{i}") for i in range(3)
        ]
        for i in range(3):
            nc.sync.dma_start(out=mp_t[i], in_=mp[i])
        mb_t = singles.tile([1, BP, 2, 256], BF16)
        nc.sync.dma_start(out=mb_t, in_=mb[:, :, :, :])

        acc = singles.tile([128, NACC], F32)
        nc.gpsimd.memset(acc, 0.0)
        onesf = singles.tile([128, 1], F32)
        nc.gpsimd.memset(onesf, 1.0)

        accP = pAcc.tile([NROW, 512], F32)

        gs = scratch.tile([128, BF], BF16, tag="gs")
        rnb_sb = scratch.tile([128, BF], BF16, tag="rnb")
        junkC = scratch.tile([128, 512], BF16, tag="junkC")
        lns = scratch.tile([128, FD], BF16, tag="lns")

        # ---------- phase A ----------
        with tc.tile_pool(name="pS", bufs=1, space="PSUM") as pS, \
             tc.tile_pool(name="pSS", bufs=1, space="PSUM") as pSS, \
             tc.tile_pool(name="pMix", bufs=2, space="PSUM") as pMix:
            sP = [pS.tile([128, 512], F32, tag=f"s{h}", name=f"s{h}")
                  for h in range(4)]
            ssP = pSS.tile([2, 512], F32)

            # HAM warm-up: dense dummy matmuls so the PE clock-gate opens
            # before the real S accumulation begins.
            for wu in range(36):
                nc.tensor.matmul(
                    out=sP[wu % 4][:, 0:128], lhsT=id_t, rhs=id_t,
                    start=True, stop=True, skip_group_check=True,
                )

            # all 8 one-hot masks up front (only need tg; DVE idles here)
            m_t = []
            for c in range(C):
                m_c = mpool.tile([128, FD], BF16, tag="m", name=f"m{c}")
                nc.vector.tensor_scalar(
                    out=m_c, in0=tg_t, scalar1=float(c), scalar2=None,
                    op0=OP.is_equal,
                )
                m_t.append(m_c)

            t_acc = None
            for c in range(C):
                q_c = qpool.tile([128, FD], BF16, tag="q")
                nc.scalar.activation(out=q_c, in_=l_tiles[c], func=AF.Exp)
                for h in range(4):
                    nc.tensor.matmul(
                        out=sP[h], lhsT=id_t,
                        rhs=q_c[:, h * 512:(h + 1) * 512],
                        start=(c == 0), stop=(c == C - 1),
                    )
                if c == 0:
                    t_acc = tpool.tile([128, FD], BF16, tag="t")
                    nc.vector.tensor_tensor(
                        out=t_acc, in0=m_t[0], in1=q_c, op=OP.mult)
                else:
                    mq = selpool.tile([128, FD], BF16, tag="mq")
                    nc.vector.tensor_tensor(
                        out=mq, in0=m_t[c], in1=q_c, op=OP.mult)
                    t_new = tpool.tile([128, FD], BF16, tag="t")
                    nc.vector.tensor_tensor(
                        out=t_new, in0=t_acc, in1=mq, op=OP.add)
                    t_acc = t_new

            # circ row-diff stencils early in the mix bank (need mk only)
            for b in range(BP):
                zP = pMix.tile([128, 512], F32, tag="mix")
                nc.tensor.matmul(
                    out=zP, lhsT=zm_t,
                    rhs=mk_t[:, b].rearrange("p c w -> p (c w)"),
                    start=True, stop=True,
                )
                nc.scalar.activation(
                    out=junkC, in_=zP, func=AF.Abs,
                    accum_out=acc[:, K_EX + b:K_EX + b + 1],
                )

            # lnS right after the loop in the ACT queue (critical path)
            for h in range(4):
                nc.scalar.activation(
                    out=lns[:, h * 512:(h + 1) * 512], in_=sP[h], func=AF.Ln)

            # squares on ScalarE (its window between lnS and lnt), sums on
            # PE into the [2, 512] norm bank
            for dc in range(4):
                sq = sqpool.tile([128, BF], BF16, tag="sq")
                nc.scalar.activation(out=sq, in_=ft_t[:, dc], func=AF.Square)
                for h in range(2):
                    nc.tensor.matmul(
                        out=ssP,
                        lhsT=oh_t[:, 31 - h:33 - h],
                        rhs=sq[:, h * 512:(h + 1) * 512],
                        start=(dc == 0 and h == 0),
                        stop=(dc == 3 and h == 1),
                        skip_group_check=True,
                    )
            # G parked in the two mix-bank buffers until rnb is ready
            gP = []
            for h in range(2):
                g = pMix.tile([128, 512], F32, tag="mix", name=f"g{h}")
                for dc in range(4):
                    nc.tensor.matmul(
                        out=g, lhsT=ft_t[:, dc, 0:128],
                        rhs=ft_t[:, dc, h * 512:(h + 1) * 512],
                        start=(dc == 0), stop=(dc == 3),
                    )
                gP.append(g)

            # rn = 1/sqrt(ss) via exp(-0.5*ln(ss)); [2,512] -> [1,1024] DMA
            lnss = tiny.tile([2, 512], F32, tag="lnss")
            nc.scalar.activation(out=lnss, in_=ssP, func=AF.Ln)
            rnrow2 = tiny.tile([2, 512], BF16, tag="rnrow2")
            nc.scalar.activation(out=rnrow2, in_=lnss, func=AF.Exp,
                                 scale=-0.5)
            rn1 = tiny.tile([1, BF], BF16, tag="rn1")
            nc.sync.dma_start(out=rn1[:, 0:512], in_=rnrow2[0:1, :])
            nc.sync.dma_start(out=rn1[:, 512:1024], in_=rnrow2[1:2, :])

            # focal tail head: lnT then ce (DVE) / p, v (ACT)
            lnt = scratch.tile([128, FD], BF16, tag="lnt")
            nc.scalar.activation(out=lnt, in_=t_acc, func=AF.Ln)
            ce = scratch.tile([128, FD], BF16, tag="ce")
            nc.vector.tensor_tensor(out=ce, in0=lns, in1=lnt, op=OP.subtract)
            p_t = scratch.tile([128, FD], BF16, tag="p")
            nc.scalar.activation(out=p_t, in_=ce, func=AF.Exp, scale=-1.0)
            v_t = scratch.tile([128, FD], BF16, tag="v")
            nc.scalar.activation(
                out=v_t, in_=p_t, func=AF.Square, scale=-1.0, bias=1.0)

        # rnb broadcast + Gs outside phase A scopes (uses its own bank)
        with tc.tile_pool(name="pR", bufs=2, space="PSUM") as pR:
            for h in range(2):
                rP = pR.tile([128, 512], F32, tag="r")
                nc.tensor.matmul(
                    out=rP, lhsT=ones_r, rhs=rn1[:, h * 512:(h + 1) * 512],
                    start=True, stop=True)
                nc.scalar.activation(
                    out=rnb_sb[:, h * 512:(h + 1) * 512], in_=rP,
                    func=AF.Copy)
                nc.vector.tensor_tensor(
                    out=gs[:, h * 512:(h + 1) * 512], in0=gP[h],
                    in1=rnb_sb[:, h * 512:(h + 1) * 512], op=OP.mult)

        # ---------- circularity col-diffs + boundary (DVE) ----------
        d_y = scratch.tile([128, BP, 2, 255], BF16, tag="dy")
        nc.vector.tensor_tensor(
            out=d_y, in0=mk_t[:, :, :, 1:256], in1=mk_t[:, :, :, 0:255],
            op=OP.subtract,
        )
        d_b = tiny.tile([1, BP, 256], BF16, tag="db")
        nc.vector.tensor_tensor(
            out=d_b, in0=mb_t[:, :, 1], in1=mb_t[:, :, 0], op=OP.subtract)
        nc.vector.tensor_reduce(
            out=acc[0:1, K_BND:K_BND + BP], in_=d_b,
            axis=AX.X, op=OP.add, apply_absolute_value=True,
        )
        junkE = scratch.tile([128, 512], BF16, tag="junkE")
        for b in range(BP):
            nc.scalar.activation(
                out=junkE[:, 0:510].rearrange("p (c w) -> p c w", c=2),
                in_=d_y[:, b], func=AF.Abs,
                accum_out=acc[:, K_EY + b:K_EY + b + 1],
            )

        # ---------- consensus + w sums (PE ones-MMs into accP rows) -----
        first = [True]

        def accmm(q, rhs, stop=False):
            n = rhs.free_size()
            chunks = [(h, min(512, n - h)) for h in range(0, n, 512)]
            for ci, (h, w) in enumerate(chunks):
                nc.tensor.matmul(
                    out=accP[:, 0:w],
                    lhsT=oh_t[:, 31 - q:63 - q],
                    rhs=rhs[:, h:h + w],
                    start=first[0],
                    stop=stop and ci == len(chunks) - 1,
                    skip_group_check=True,
                )
                first[0] = False

        for i in range(3):
            accmm(R_S + i, mp_t[i])
        for b in range(BP):
            accmm(R_AREA + b, mk_t[:, b].rearrange("p c w -> p (c w)"))
        for k, (i, j) in enumerate(((0, 1), (0, 2), (1, 2))):
            pr = prodpool.tile([128, FD], BF16, tag="prod")
            nc.vector.tensor_tensor(
                out=pr, in0=mp_t[i], in1=mp_t[j], op=OP.mult)
            accmm(R_I + k, pr)

        # focal w + its accP rows
        w_t = scratch.tile([128, FD], BF16, tag="w")
        nc.vector.tensor_tensor(out=w_t, in0=v_t, in1=ce, op=OP.mult)
        accmm(R_W, w_t, stop=True)

        # ---------- contrastive tail (SBUF only) ----------
        rdj = scratch.tile([128, 128], BF16, tag="rdj")
        nc.vector.tensor_tensor(
            out=rdj, in0=rnb_sb[:, 0:128], in1=id_t, op=OP.mult)
        rn_r = tiny.tile([128, 1], F32, tag="rnr")
        nc.vector.tensor_reduce(
            out=rn_r, in_=rdj, axis=AX.X, op=OP.add)
        gq = scratch.tile([128, BF], BF16, tag="gq")
        nc.vector.tensor_scalar(
            out=gq, in0=gs, scalar1=rn_r, scalar2=None, op0=OP.mult)
        pj = scratch.tile([128, 128], BF16, tag="pj")
        nc.vector.tensor_tensor(
            out=pj, in0=gq[:, 512:640], in1=id_t, op=OP.mult)
        posu = tiny.tile([128, 1], F32, tag="posu")
        nc.vector.tensor_reduce(out=posu, in_=pj, axis=AX.X, op=OP.add)
        nc.vector.scalar_tensor_tensor(
            out=gq[:, 0:128], in0=id_t, scalar=-1e4, in1=gq[:, 0:128],
            op0=OP.mult, op1=OP.add,
        )
        esum = tiny.tile([128, 1], F32, tag="esum")
        junkB = scratch.tile([128, BF], BF16, tag="junkB")
        nc.scalar.activation(
            out=junkB, in_=gq, func=AF.Exp, scale=1.0 / TEMP, accum_out=esum)
        lse = tiny.tile([128, 1], F32, tag="lse")
        nc.scalar.activation(out=lse, in_=esum, func=AF.Ln)
        nc.vector.scalar_tensor_tensor(
            out=acc[:, K_CONTRAST:K_CONTRAST + 1], in0=posu,
            scalar=-1.0 / TEMP, in1=lse, op0=OP.mult, op1=OP.add,
        )

        # ---------- finalize ----------
        with tc.tile_pool(name="pF", bufs=1, space="PSUM") as pF:
            junkD = scratch.tile([NROW, 512], BF16, tag="junkD")
            pb_sb = tiny.tile([NROW, 1], F32, tag="pbs")
            nc.scalar.activation(
                out=junkD, in_=accP, func=AF.Copy, accum_out=pb_sb)
            nc.sync.dma_start(out=pb[:, :], in_=pb_sb)

            pfin = pF.tile([1, NACC], F32)
            nc.tensor.matmul(
                out=pfin, lhsT=onesf, rhs=acc, start=True, stop=True)
            pa_sb = tiny.tile([1, NACC], F32, tag="pas")
            nc.vector.tensor_copy(out=pa_sb, in_=pfin)
            nc.sync.dma_start(out=pa[:, :], in_=pa_sb)


def _zmat():
    ident = np.eye(128, dtype=np.float32)
    z = np.roll(ident, -1, axis=0) - ident
    z[:, 127] = 0.0
    return np.ascontiguousarray(z)


def _host_inputs(logits, target, features, masks, method_preds):
    """Slice/reshape/cast full inputs into per-core input maps (bf16)."""
    bf = ml_dtypes.bfloat16
    ident = np.eye(128, dtype=np.float32)
    ohb = np.zeros((128, 63), dtype=np.float32)
    ohb[:, 31] = 1.0
    consts = {
        "idb": ident.astype(bf),
        "zmb": _zmat().astype(bf),
        "ohb": ohb.astype(bf),
        "onesr": np.ones((1, 128), dtype=np.float32).astype(bf),
    }
    in_maps = []
    for c in range(NCORES):
        b0 = c * BP
        lgc = (logits[b0:b0 + BP].reshape(BP, C, 128, 512)
               .transpose(1, 2, 0, 3).reshape(C, 128, FD))
        tgc = (target[b0:b0 + BP].reshape(BP, 128, 512)
               .transpose(1, 0, 2).reshape(128, FD))
        mkc = (masks[b0:b0 + BP, 0].reshape(BP, 2, 128, 256)
               .transpose(2, 0, 1, 3))
        mbc = masks[b0:b0 + BP, 0, 127:129, :].reshape(1, BP, 2, 256)
        mpc = (method_preds[:, b0:b0 + BP].reshape(3, BP, 128, 512)
               .transpose(0, 2, 1, 3).reshape(3, 128, FD))
        ftc = (np.roll(features, -c * 128, axis=0).T
               .reshape(4, 128, BF).transpose(1, 0, 2))
        in_maps.append({
            "lg": np.ascontiguousarray(lgc).astype(ml_dtypes.float8_e4m3fn),
            "tg": np.ascontiguousarray(tgc.astype(np.float32)).astype(bf),
            "mk": np.ascontiguousarray(mkc).astype(bf),
            "mb": np.ascontiguousarray(mbc).astype(bf),
            "mp": np.ascontiguousarray(mpc).astype(bf),
            "ft": np.ascontiguousarray(ftc).astype(bf),
            **consts,
        })
    return in_maps


def _combine(pas, pbs):
    """Host-side combination of the per-core partial vectors."""
    PA = np.stack([np.asarray(p).reshape(-1).astype(np.float64)
                   for p in pas])  # [8, NACC]
    PB = np.stack([np.asarray(p).reshape(-1).astype(np.float64)
                   for p in pbs])  # [8, NROW]

    focal = GAMMA_SCALE * PB[:, R_W].sum() / (B * HW)
    contrast = 0.5 * PA[:, K_CONTRAST].sum() / BF

    circ_total = 0.0
    for c in range(NCORES):
        for b in range(BP):
            area = PB[c, R_AREA + b]
            ex = PA[c, K_EX + b] + PA[c, K_BND + b]
            ey = PA[c, K_EY + b]
            per = ex + ey
            if area > 0 and per > 0:
                circv = 4.0 * np.pi * area / max(per, 1e-12) ** 2
                circ_total += (circv - 1.0) ** 2
    circ = 0.1 * circ_total / B

    S = PB[:, R_S:R_S + 3].sum(axis=0)
    I = PB[:, R_I:R_I + 3].sum(axis=0)
    cons_total = 0.0
    for k, (i, j) in enumerate(((0, 1), (0, 2), (1, 2))):
        union = S[i] + S[j] - I[k]
        iou = I[k] / (union + 1e-6)
        cons_total += max(0.6 - iou, 0.0)
    consensus = 0.3 * cons_total / 3.0

    return np.float32(focal + contrast + circ + consensus)


_CACHED_NC = None


def _get_nc():
    global _CACHED_NC
    if _CACHED_NC is None:
        _CACHED_NC = _build_nc()
    return _CACHED_NC


def kernel(logits, target, features, masks, method_preds):
    logits = np.asarray(logits, dtype=np.float32)
    target = np.asarray(target, dtype=np.int32)
    features = np.asarray(features, dtype=np.float32)
    masks = np.asarray(masks, dtype=np.float32)
    method_preds = np.asarray(method_preds, dtype=np.float32)

    in_maps = _host_inputs(logits, target, features, masks, method_preds)
    res = run_bass_kernel_spmd(_get_nc(), in_maps, list(range(NCORES)))
    pas = [res.results[c]["pa"] for c in range(NCORES)]
    pbs = [res.results[c]["pb"] for c in range(NCORES)]
    return _combine(pas, pbs)
